# revision 8
# baseline (speedup 1.0000x reference)
"""Trainium2 Bass kernel for nn_DKSTE_85315230367936 (embedding_lookup).

Math (per batch element b, dim d, K=2 planes):
    x = sign(rel[b,d,0]); y = sign(rel[b,d,1]); a = sign(alpha[b,d])
    s = (x+y)/2 ; dd = (x-y)/2
    term = s*(h0*t0 + a*h1*t1) + dd*(h1*t0 - a*h0*t1)
    out[b] = sqrt(sum_d term^2)

Key identity: s*dd == 0 elementwise (x,y are +-1), so with s2 = x+y,
d2 = x-y:
    (2*term)^2 = (s2*A)^2 + (d2*B)^2   where A = h0*t0 + a*h1*t1,
                                             B = h1*t0 - a*h0*t1
(the cross term vanishes exactly).  u = s2*A and w = d2*B are written into
one [u|w] tile so ScalarE accumulates u^2+w^2 in a single Square pass per
tile; the final sqrt rescales by 0.25.

Strategy: pure batch data-parallelism (1024 elements/core), entity table
replicated in every core's HBM as one [200000, 1024] fp16 table whose rows
are [k=0 plane | k=1 plane].  Per core:
  1. sign-table precompute on device: s2, d2, a packed per relation as one
     fp16 [500, 1536] DRAM table ([s2|d2|a] rows).
  2. batch processed in 4 chunks of 2 tiles (2x128 elements).  All row
     gathers are [128,1]-offset indirect DMAs (the only offset layout the
     Q7 ucode supports - wider offset tensors silently gather consecutive
     rows): per chunk 2 head + 2 tail + 2 sign-table igathers, round-robin
     over the 4 SWDGE queues so 3+ queues drain concurrently (~360 GB/s
     aggregate vs ~150 GB/s for a single queue).  Gathers are issued two
     chunks ahead of compute.
  3. VectorE runs 9 fp16 tensor_tensor ops per chunk on [128,2,512]
     strided views (2 tiles fused per op to amortize the ~151-cycle DVE
     instruction setup).
  4. ScalarE: one Square+accumulate per tile over the packed [u|w] pair,
     final sqrt(0.25*(sum)).
Output [128, 8] f32 per core; host inverse-permutes to [8192].
"""

import sys

for _p in ("/opt/trn_rl_repo",):
    if _p not in sys.path:
        sys.path.insert(0, _p)

import numpy as np

import concourse.bass as bass
import concourse.bacc as bacc
import concourse.tile as tile
from concourse import mybir
from concourse.bass_utils import run_bass_kernel_spmd

NENTITY, NRELATION, EMB_DIM, K = 200000, 500, 512, 2
BATCH = 8192
NCORES = 8
B_LOC = BATCH // NCORES            # 1024 batch elements per core
NT = B_LOC // 128                  # 8 tiles of 128 per core
CH = 4                             # chunks per core
T_C = NT // CH                     # tiles per chunk (2)
CDT = mybir.dt.float16
NP_CDT = np.float16

F32 = mybir.dt.float32
F8 = mybir.dt.float8e4
I32 = mybir.dt.int32
AF = mybir.ActivationFunctionType
ALU = mybir.AluOpType

REL_P = 125
RROW = NRELATION // REL_P                    # 4 relation rows per partition
REL_FREE = NRELATION * EMB_DIM // REL_P      # 2048 (per plane)
SDA_W = 3 * EMB_DIM                          # 1536: [s2 | d2 | a]
LOOKAHEAD = 2                                # chunks of gather prefetch


def build_program():
    nc = bacc.Bacc("TRN2", target_bir_lowering=False, debug=False,
                   num_swdge_queues=4)

    ea = nc.declare_dram_parameter("ea", [NENTITY, 2 * EMB_DIM], F8, isOutput=False)
    relx = nc.declare_dram_parameter("relx", [REL_P, REL_FREE], CDT, isOutput=False)
    rely = nc.declare_dram_parameter("rely", [REL_P, REL_FREE], CDT, isOutput=False)
    alphaf = nc.declare_dram_parameter("alphaf", [REL_P, REL_FREE], CDT, isOutput=False)
    htidx = nc.declare_dram_parameter("htidx", [128, 2 * NT], I32, isOutput=False)
    relidx = nc.declare_dram_parameter("relidx", [128, NT], I32, isOutput=False)
    out = nc.declare_dram_parameter("out", [128, NT], F32, isOutput=True)

    with tile.TileContext(nc) as tc:
        with (
            tc.tile_pool(name="dram", bufs=1, space="DRAM") as dramp,
            tc.tile_pool(name="idx", bufs=1) as idxp,
            tc.tile_pool(name="prep", bufs=1) as prep,
            tc.tile_pool(name="gat", bufs=4) as gat,
            tc.tile_pool(name="wrk", bufs=2) as wrk,
            tc.tile_pool(name="outp", bufs=1) as outp,
        ):
            # internal DRAM: per-relation [s2 | d2 | a] rows of 3*512 fp8
            # (sign values are exact in e4m3; halves write+gather bytes, the
            # cast back to fp16 happens inside the gather DMA)
            sda = dramp.tile([NRELATION, SDA_W], F8)

            # ---- index upload -------------------------------------------
            ht_t = idxp.tile([128, 2 * NT], I32)
            nc.sync.dma_start(out=ht_t[:], in_=htidx[:])
            rel_t = idxp.tile([128, NT], I32)
            nc.sync.dma_start(out=rel_t[:], in_=relidx[:])

            qn = [0]

            def igather(out_ap, in_ap, off_ap):
                inst = nc.gpsimd.indirect_dma_start(
                    out=out_ap, out_offset=None, in_=in_ap,
                    in_offset=bass.IndirectOffsetOnAxis(ap=off_ap, axis=0),
                )
                q = qn[0] % 3
                qn[0] += 1
                if q:
                    inst.ins.queue = f"qPoolDynamic{q}"
                return inst

            hts = [None] * CH
            tts = [None] * CH
            gss = [None] * CH

            def issue_ht_gathers(c):
                gh = gat.tile([128, T_C, 2 * EMB_DIM], CDT, tag="gh")
                gt = gat.tile([128, T_C, 2 * EMB_DIM], CDT, tag="gt")
                for j in range(T_C):
                    t_idx = T_C * c + j
                    igather(gh[:, j, :], ea[:], ht_t[:, 2 * t_idx : 2 * t_idx + 1])
                    igather(gt[:, j, :], ea[:], ht_t[:, 2 * t_idx + 1 : 2 * t_idx + 2])
                hts[c] = gh
                tts[c] = gt

            def issue_sda_gathers(c):
                gs = gat.tile([128, T_C, SDA_W], CDT, tag="gs")
                for j in range(T_C):
                    t_idx = T_C * c + j
                    igather(gs[:, j, :], sda[:], rel_t[:, t_idx : t_idx + 1])
                gss[c] = gs

            # ---- sign-table precompute ----------------------------------
            rxsb = prep.tile([REL_P, REL_FREE], CDT)
            nc.scalar.dma_start(out=rxsb[:], in_=relx[:])
            rysb = prep.tile([REL_P, REL_FREE], CDT)
            i_ry = nc.gpsimd.dma_start(out=rysb[:], in_=rely[:])
            i_ry.ins.queue = "qPoolDynamic3"
            alsb = prep.tile([REL_P, REL_FREE], CDT)
            i_al = nc.gpsimd.dma_start(out=alsb[:], in_=alphaf[:])
            i_al.ins.queue = "qPoolDynamic3"
            sx = prep.tile([REL_P, REL_FREE], CDT)
            nc.scalar.activation(sx[:], rxsb[:], AF.Sign)
            sy = prep.tile([REL_P, REL_FREE], CDT)
            nc.scalar.activation(sy[:], rysb[:], AF.Sign)
            sda_sb = prep.tile([REL_P, RROW * SDA_W], CDT)
            sda_sbv = sda_sb[:].rearrange("p (r c d) -> p r c d", c=3, d=EMB_DIM)
            sx3 = sx[:].rearrange("p (r d) -> p r d", d=EMB_DIM)
            sy3 = sy[:].rearrange("p (r d) -> p r d", d=EMB_DIM)
            nc.vector.tensor_tensor(
                out=sda_sbv[:, :, 0, :], in0=sx3, in1=sy3, op=ALU.add
            )
            nc.vector.tensor_tensor(
                out=sda_sbv[:, :, 1, :], in0=sx3, in1=sy3, op=ALU.subtract
            )
            nc.scalar.activation(
                sda_sbv[:, :, 2, :],
                alsb[:].rearrange("p (r d) -> p r d", d=EMB_DIM),
                AF.Sign,
            )

            # preload the Sqrt LUT during the precompute window so the final
            # sqrt doesn't pay the ACT table swap on the critical tail
            sq_dummy = outp.tile([128, 1], F32)
            nc.vector.memset(sq_dummy[:], 1.0)
            nc.scalar.activation(sq_dummy[:], sq_dummy[:], AF.Sqrt)

            issue_ht_gathers(0)
            issue_ht_gathers(1)
            i_wr = nc.gpsimd.dma_start(
                out=sda[:].rearrange("(p r) w -> p (r w)", r=RROW), in_=sda_sb[:]
            )
            i_wr.ins.queue = "qPoolDynamic3"
            issue_sda_gathers(0)
            issue_sda_gathers(1)
            issue_ht_gathers(2)
            issue_sda_gathers(2)
            issue_ht_gathers(3)
            issue_sda_gathers(3)

            # ---- main loop ----------------------------------------------
            ssum = outp.tile([128, NT], F32)
            junk = outp.tile([128, 2 * EMB_DIM], CDT)

            def tt(tag, in0, in1, op, out_ap=None):
                if out_ap is None:
                    t = wrk.tile([128, T_C, EMB_DIM], CDT, tag=tag)
                    out_ap = t[:]
                nc.vector.tensor_tensor(out=out_ap, in0=in0, in1=in1, op=op)
                return out_ap

            for c in range(CH):
                gh3 = hts[c][:]
                gt3 = tts[c][:]
                gs3 = gss[c][:]
                h0 = gh3[:, :, 0:EMB_DIM]
                h1 = gh3[:, :, EMB_DIM : 2 * EMB_DIM]
                t0 = gt3[:, :, 0:EMB_DIM]
                t1v = gt3[:, :, EMB_DIM : 2 * EMB_DIM]
                s2v = gs3[:, :, 0:EMB_DIM]
                d2v = gs3[:, :, EMB_DIM : 2 * EMB_DIM]
                av = gs3[:, :, 2 * EMB_DIM : 3 * EMB_DIM]

                uw_t = wrk.tile([128, T_C, 2 * EMB_DIM], CDT, tag="uw")
                uw = uw_t[:]

                t1p = tt("t1p", av, t1v, ALU.mult)
                m1 = tt("m1", h0, t0, ALU.mult)
                m2 = tt("m2", h1, t1p, ALU.mult)
                A = tt("A", m1, m2, ALU.add)
                m4 = tt("m4", h1, t0, ALU.mult)
                m5 = tt("m5", h0, t1p, ALU.mult)
                B = tt("B", m4, m5, ALU.subtract)
                tt("u", s2v, A, ALU.mult, out_ap=uw[:, :, 0:EMB_DIM])
                tt("w", d2v, B, ALU.mult, out_ap=uw[:, :, EMB_DIM : 2 * EMB_DIM])

                for j in range(T_C):
                    t_idx = T_C * c + j
                    nc.scalar.activation(
                        junk[:], uw[:, j, :], AF.Square,
                        accum_out=ssum[:, t_idx : t_idx + 1],
                    )


            res = outp.tile([128, NT], F32)
            # score = sqrt(sum(term^2)) = sqrt(0.25 * sum((2*term)^2))
            nc.scalar.activation(res[:], ssum[:], AF.Sqrt, scale=0.25)
            nc.sync.dma_start(out=out[:], in_=res[:])

    nc.compile()
    return nc


_NC_CACHE = None


def _get_program():
    global _NC_CACHE
    if _NC_CACHE is None:
        _NC_CACHE = build_program()
    return _NC_CACHE


def make_in_maps(head_idx, relation_idx, tail_idx, entity_embedding,
                 relation_embedding, alpha_embedding):
    """Host-side sharding: slice batch 1024/core, replicate tables."""
    head_idx = np.asarray(head_idx).astype(np.int32)
    relation_idx = np.asarray(relation_idx).astype(np.int32)
    tail_idx = np.asarray(tail_idx).astype(np.int32)
    ent = np.asarray(entity_embedding)
    rel = np.asarray(relation_embedding)
    alp = np.asarray(alpha_embedding)

    import ml_dtypes

    # ea row r = [E[r,:,0,0] | E[r,:,0,1]], stored e4m3 (decoded to fp16 by
    # the gather DMA; max output rel err ~0.94% vs the 2e-2 gate)
    ea = np.ascontiguousarray(
        ent[:, :, 0, :].transpose(0, 2, 1).reshape(NENTITY, 2 * EMB_DIM)
    ).astype(ml_dtypes.float8_e4m3).view(np.uint8)
    relx = np.ascontiguousarray(rel[:, :, 0]).astype(NP_CDT).reshape(REL_P, REL_FREE)
    rely = np.ascontiguousarray(rel[:, :, 1]).astype(NP_CDT).reshape(REL_P, REL_FREE)
    alphaf = alp.astype(NP_CDT).reshape(REL_P, REL_FREE)

    in_maps = []
    for c_core in range(NCORES):
        lo = c_core * B_LOC
        h = head_idx[lo : lo + B_LOC]
        tl = tail_idx[lo : lo + B_LOC]
        r = relation_idx[lo : lo + B_LOC]
        # htidx[p, 2t] = head of elem t*128+p ; [p, 2t+1] = tail
        htp = np.empty((128, 2 * NT), np.int32)
        for t in range(NT):
            htp[:, 2 * t] = h[128 * t : 128 * (t + 1)]
            htp[:, 2 * t + 1] = tl[128 * t : 128 * (t + 1)]
        rlp = np.empty((128, NT), np.int32)
        for t in range(NT):
            rlp[:, t] = r[128 * t : 128 * (t + 1)]
        in_maps.append(
            {
                "ea": ea,
                "relx": relx,
                "rely": rely,
                "alphaf": alphaf,
                "htidx": htp,
                "relidx": rlp,
            }
        )
    return in_maps


def unshard_out(results):
    """results: list of per-core dicts with 'out' [128, NT] f32."""
    full = np.empty(BATCH, np.float32)
    for c in range(NCORES):
        o = np.asarray(results[c]["out"])          # [128, NT], col = t
        # elem 128t + p  <-  o[p, t]
        full[c * B_LOC : (c + 1) * B_LOC] = o.T.ravel()
    return full


def kernel(head_idx, relation_idx, tail_idx, entity_embedding,
           relation_embedding, alpha_embedding):
    nc = _get_program()
    in_maps = make_in_maps(head_idx, relation_idx, tail_idx, entity_embedding,
                           relation_embedding, alpha_embedding)
    res = run_bass_kernel_spmd(nc, in_maps, list(range(NCORES)))
    return unshard_out(res.results)


# revision 10
# speedup vs baseline: 1.1040x; 1.1040x over previous
"""Trainium2 Bass kernel for nn_DKSTE_85315230367936 (embedding_lookup).

Math (per batch element b, dim d, K=2 planes):
    x = sign(rel[b,d,0]); y = sign(rel[b,d,1]); a = sign(alpha[b,d])
    s = (x+y)/2 ; dd = (x-y)/2
    term = s*(h0*t0 + a*h1*t1) + dd*(h1*t0 - a*h0*t1)
    out[b] = sqrt(sum_d term^2)

Key identity: s*dd == 0 elementwise (x,y are +-1), so with s2 = x+y,
d2 = x-y:
    (2*term)^2 = (s2*A)^2 + (d2*B)^2   where A = h0*t0 + a*h1*t1,
                                             B = h1*t0 - a*h0*t1
(the cross term vanishes exactly).  u = s2*A and w = d2*B are written into
one [u|w] tile so ScalarE accumulates u^2+w^2 in a single Square pass per
tile; the final sqrt rescales by 0.25.

Strategy: pure batch data-parallelism (1024 elements/core), entity table
replicated in every core's HBM as one [200000, 1024] fp16 table whose rows
are [k=0 plane | k=1 plane].  Per core:
  1. sign-table precompute on device: s2, d2, a packed per relation as one
     fp16 [500, 1536] DRAM table ([s2|d2|a] rows).
  2. batch processed in 4 chunks of 2 tiles (2x128 elements).  All row
     gathers are [128,1]-offset indirect DMAs (the only offset layout the
     Q7 ucode supports - wider offset tensors silently gather consecutive
     rows): per chunk 2 head + 2 tail + 2 sign-table igathers, round-robin
     over the 4 SWDGE queues so 3+ queues drain concurrently (~360 GB/s
     aggregate vs ~150 GB/s for a single queue).  Gathers are issued two
     chunks ahead of compute.
  3. VectorE runs 9 fp16 tensor_tensor ops per chunk on [128,2,512]
     strided views (2 tiles fused per op to amortize the ~151-cycle DVE
     instruction setup).
  4. ScalarE: one Square+accumulate per tile over the packed [u|w] pair,
     final sqrt(0.25*(sum)).
Output [128, 8] f32 per core; host inverse-permutes to [8192].
"""

import sys

for _p in ("/opt/trn_rl_repo",):
    if _p not in sys.path:
        sys.path.insert(0, _p)

import numpy as np

import concourse.bass as bass
import concourse.bacc as bacc
import concourse.tile as tile
from concourse import mybir
from concourse.bass_utils import run_bass_kernel_spmd

NENTITY, NRELATION, EMB_DIM, K = 200000, 500, 512, 2
BATCH = 8192
NCORES = 8
B_LOC = BATCH // NCORES            # 1024 batch elements per core
NT = B_LOC // 128                  # 8 tiles of 128 per core
CH = 4                             # chunks per core
T_C = NT // CH                     # tiles per chunk (2)
CDT = mybir.dt.float16
NP_CDT = np.float16

F32 = mybir.dt.float32
F8 = mybir.dt.float8e4
I32 = mybir.dt.int32
AF = mybir.ActivationFunctionType
ALU = mybir.AluOpType

REL_P = 125
RROW = NRELATION // REL_P                    # 4 relation rows per partition
REL_FREE = NRELATION * EMB_DIM // REL_P      # 2048 (per plane)
SDA_W = 3 * EMB_DIM                          # 1536: [s2 | d2 | a]
LOOKAHEAD = 2                                # chunks of gather prefetch


def build_program():
    nc = bacc.Bacc("TRN2", target_bir_lowering=False, debug=False,
                   num_swdge_queues=4, dynamic_dma_scratch_size=49152)

    ea = nc.declare_dram_parameter("ea", [NENTITY, 2 * EMB_DIM], CDT, isOutput=False)
    relx = nc.declare_dram_parameter("relx", [REL_P, REL_FREE], CDT, isOutput=False)
    rely = nc.declare_dram_parameter("rely", [REL_P, REL_FREE], CDT, isOutput=False)
    alphaf = nc.declare_dram_parameter("alphaf", [REL_P, REL_FREE], CDT, isOutput=False)
    htidx = nc.declare_dram_parameter("htidx", [128, 2 * NT], I32, isOutput=False)
    relidx = nc.declare_dram_parameter("relidx", [128, NT], I32, isOutput=False)
    out = nc.declare_dram_parameter("out", [128, NT], F32, isOutput=True)

    with tile.TileContext(nc) as tc:
        with (
            tc.tile_pool(name="dram", bufs=1, space="DRAM") as dramp,
            tc.tile_pool(name="idx", bufs=1) as idxp,
            tc.tile_pool(name="prep", bufs=1) as prep,
            tc.tile_pool(name="gat", bufs=4) as gat,
            tc.tile_pool(name="wrk", bufs=2) as wrk,
            tc.tile_pool(name="outp", bufs=1) as outp,
        ):
            # internal DRAM: per-relation [s2 | d2 | a] rows of 3*512 fp8
            # (sign values are exact in e4m3; halves write+gather bytes, the
            # cast back to fp16 happens inside the gather DMA)
            sda = dramp.tile([NRELATION, SDA_W], F8)

            # ---- index upload -------------------------------------------
            ht_t = idxp.tile([128, 2 * NT], I32)
            nc.sync.dma_start(out=ht_t[:], in_=htidx[:])
            rel_t = idxp.tile([128, NT], I32)
            nc.sync.dma_start(out=rel_t[:], in_=relidx[:])

            qn = [0]

            def igather(out_ap, in_ap, off_ap):
                inst = nc.gpsimd.indirect_dma_start(
                    out=out_ap, out_offset=None, in_=in_ap,
                    in_offset=bass.IndirectOffsetOnAxis(ap=off_ap, axis=0),
                )
                q = qn[0] % 3
                qn[0] += 1
                if q:
                    inst.ins.queue = f"qPoolDynamic{q}"
                return inst

            hts = [None] * CH
            tts = [None] * CH
            gss = [None] * CH

            def issue_ht_gathers(c):
                gh = gat.tile([128, T_C, 2 * EMB_DIM], CDT, tag="gh")
                gt = gat.tile([128, T_C, 2 * EMB_DIM], CDT, tag="gt")
                for j in range(T_C):
                    t_idx = T_C * c + j
                    igather(gh[:, j, :], ea[:], ht_t[:, 2 * t_idx : 2 * t_idx + 1])
                    igather(gt[:, j, :], ea[:], ht_t[:, 2 * t_idx + 1 : 2 * t_idx + 2])
                hts[c] = gh
                tts[c] = gt

            def issue_sda_gathers(c):
                gs = gat.tile([128, T_C, SDA_W], CDT, tag="gs")
                for j in range(T_C):
                    t_idx = T_C * c + j
                    igather(gs[:, j, :], sda[:], rel_t[:, t_idx : t_idx + 1])
                gss[c] = gs

            # ---- sign-table precompute ----------------------------------
            rxsb = prep.tile([REL_P, REL_FREE], CDT)
            nc.scalar.dma_start(out=rxsb[:], in_=relx[:])
            rysb = prep.tile([REL_P, REL_FREE], CDT)
            i_ry = nc.gpsimd.dma_start(out=rysb[:], in_=rely[:])
            i_ry.ins.queue = "qPoolDynamic3"
            alsb = prep.tile([REL_P, REL_FREE], CDT)
            i_al = nc.gpsimd.dma_start(out=alsb[:], in_=alphaf[:])
            i_al.ins.queue = "qPoolDynamic3"
            sx = prep.tile([REL_P, REL_FREE], CDT)
            nc.scalar.activation(sx[:], rxsb[:], AF.Sign)
            sy = prep.tile([REL_P, REL_FREE], CDT)
            nc.scalar.activation(sy[:], rysb[:], AF.Sign)
            sda_sb = prep.tile([REL_P, RROW * SDA_W], CDT)
            sda_sbv = sda_sb[:].rearrange("p (r c d) -> p r c d", c=3, d=EMB_DIM)
            sx3 = sx[:].rearrange("p (r d) -> p r d", d=EMB_DIM)
            sy3 = sy[:].rearrange("p (r d) -> p r d", d=EMB_DIM)
            nc.vector.tensor_tensor(
                out=sda_sbv[:, :, 0, :], in0=sx3, in1=sy3, op=ALU.add
            )
            nc.vector.tensor_tensor(
                out=sda_sbv[:, :, 1, :], in0=sx3, in1=sy3, op=ALU.subtract
            )
            nc.scalar.activation(
                sda_sbv[:, :, 2, :],
                alsb[:].rearrange("p (r d) -> p r d", d=EMB_DIM),
                AF.Sign,
            )

            # preload the Sqrt LUT during the precompute window so the final
            # sqrt doesn't pay the ACT table swap on the critical tail
            sq_dummy = outp.tile([128, 1], F32)
            nc.vector.memset(sq_dummy[:], 1.0)
            nc.scalar.activation(sq_dummy[:], sq_dummy[:], AF.Sqrt)

            issue_ht_gathers(0)
            issue_ht_gathers(1)
            i_wr = nc.gpsimd.dma_start(
                out=sda[:].rearrange("(p r) w -> p (r w)", r=RROW), in_=sda_sb[:]
            )
            i_wr.ins.queue = "qPoolDynamic3"
            issue_sda_gathers(0)
            issue_sda_gathers(1)
            issue_ht_gathers(2)
            issue_sda_gathers(2)
            issue_ht_gathers(3)
            issue_sda_gathers(3)

            # ---- main loop ----------------------------------------------
            ssum = outp.tile([128, NT], F32)
            junk = outp.tile([128, 2 * EMB_DIM], CDT)

            def tt(tag, in0, in1, op, out_ap=None):
                if out_ap is None:
                    t = wrk.tile([128, T_C, EMB_DIM], CDT, tag=tag)
                    out_ap = t[:]
                nc.vector.tensor_tensor(out=out_ap, in0=in0, in1=in1, op=op)
                return out_ap

            for c in range(CH):
                gh3 = hts[c][:]
                gt3 = tts[c][:]
                gs3 = gss[c][:]
                h0 = gh3[:, :, 0:EMB_DIM]
                h1 = gh3[:, :, EMB_DIM : 2 * EMB_DIM]
                t0 = gt3[:, :, 0:EMB_DIM]
                t1v = gt3[:, :, EMB_DIM : 2 * EMB_DIM]
                s2v = gs3[:, :, 0:EMB_DIM]
                d2v = gs3[:, :, EMB_DIM : 2 * EMB_DIM]
                av = gs3[:, :, 2 * EMB_DIM : 3 * EMB_DIM]

                uw_t = wrk.tile([128, T_C, 2 * EMB_DIM], CDT, tag="uw")
                uw = uw_t[:]

                t1p = tt("t1p", av, t1v, ALU.mult)
                m1 = tt("m1", h0, t0, ALU.mult)
                m2 = tt("m2", h1, t1p, ALU.mult)
                A = tt("A", m1, m2, ALU.add)
                m4 = tt("m4", h1, t0, ALU.mult)
                m5 = tt("m5", h0, t1p, ALU.mult)
                B = tt("B", m4, m5, ALU.subtract)
                tt("u", s2v, A, ALU.mult, out_ap=uw[:, :, 0:EMB_DIM])
                tt("w", d2v, B, ALU.mult, out_ap=uw[:, :, EMB_DIM : 2 * EMB_DIM])

                for j in range(T_C):
                    t_idx = T_C * c + j
                    nc.scalar.activation(
                        junk[:], uw[:, j, :], AF.Square,
                        accum_out=ssum[:, t_idx : t_idx + 1],
                    )


            res = outp.tile([128, NT], F32)
            # score = sqrt(sum(term^2)) = sqrt(0.25 * sum((2*term)^2))
            nc.scalar.activation(res[:], ssum[:], AF.Sqrt, scale=0.25)
            nc.sync.dma_start(out=out[:], in_=res[:])

    nc.compile()
    return nc


_NC_CACHE = None


def _get_program():
    global _NC_CACHE
    if _NC_CACHE is None:
        _NC_CACHE = build_program()
    return _NC_CACHE


def make_in_maps(head_idx, relation_idx, tail_idx, entity_embedding,
                 relation_embedding, alpha_embedding):
    """Host-side sharding: slice batch 1024/core, replicate tables."""
    head_idx = np.asarray(head_idx).astype(np.int32)
    relation_idx = np.asarray(relation_idx).astype(np.int32)
    tail_idx = np.asarray(tail_idx).astype(np.int32)
    ent = np.asarray(entity_embedding)
    rel = np.asarray(relation_embedding)
    alp = np.asarray(alpha_embedding)

    # ea row r = [E[r,:,0,0] | E[r,:,0,1]]
    ea = np.ascontiguousarray(
        ent[:, :, 0, :].transpose(0, 2, 1).reshape(NENTITY, 2 * EMB_DIM)
    ).astype(NP_CDT)
    relx = np.ascontiguousarray(rel[:, :, 0]).astype(NP_CDT).reshape(REL_P, REL_FREE)
    rely = np.ascontiguousarray(rel[:, :, 1]).astype(NP_CDT).reshape(REL_P, REL_FREE)
    alphaf = alp.astype(NP_CDT).reshape(REL_P, REL_FREE)

    in_maps = []
    for c_core in range(NCORES):
        lo = c_core * B_LOC
        h = head_idx[lo : lo + B_LOC]
        tl = tail_idx[lo : lo + B_LOC]
        r = relation_idx[lo : lo + B_LOC]
        # htidx[p, 2t] = head of elem t*128+p ; [p, 2t+1] = tail
        htp = np.empty((128, 2 * NT), np.int32)
        for t in range(NT):
            htp[:, 2 * t] = h[128 * t : 128 * (t + 1)]
            htp[:, 2 * t + 1] = tl[128 * t : 128 * (t + 1)]
        rlp = np.empty((128, NT), np.int32)
        for t in range(NT):
            rlp[:, t] = r[128 * t : 128 * (t + 1)]
        in_maps.append(
            {
                "ea": ea,
                "relx": relx,
                "rely": rely,
                "alphaf": alphaf,
                "htidx": htp,
                "relidx": rlp,
            }
        )
    return in_maps


def unshard_out(results):
    """results: list of per-core dicts with 'out' [128, NT] f32."""
    full = np.empty(BATCH, np.float32)
    for c in range(NCORES):
        o = np.asarray(results[c]["out"])          # [128, NT], col = t
        # elem 128t + p  <-  o[p, t]
        full[c * B_LOC : (c + 1) * B_LOC] = o.T.ravel()
    return full


def kernel(head_idx, relation_idx, tail_idx, entity_embedding,
           relation_embedding, alpha_embedding):
    nc = _get_program()
    in_maps = make_in_maps(head_idx, relation_idx, tail_idx, entity_embedding,
                           relation_embedding, alpha_embedding)
    res = run_bass_kernel_spmd(nc, in_maps, list(range(NCORES)))
    return unshard_out(res.results)


# revision 12
# speedup vs baseline: 1.1310x; 1.0245x over previous
"""Trainium2 Bass kernel for nn_DKSTE_85315230367936 (embedding_lookup).

Math (per batch element b, dim d, K=2 planes):
    x = sign(rel[b,d,0]); y = sign(rel[b,d,1]); a = sign(alpha[b,d])
    s = (x+y)/2 ; dd = (x-y)/2
    term = s*(h0*t0 + a*h1*t1) + dd*(h1*t0 - a*h0*t1)
    out[b] = sqrt(sum_d term^2)

Key identity: s*dd == 0 elementwise (x,y are +-1), so with s2 = x+y,
d2 = x-y:
    (2*term)^2 = (s2*A)^2 + (d2*B)^2   where A = h0*t0 + a*h1*t1,
                                             B = h1*t0 - a*h0*t1
(the cross term vanishes exactly).  u = s2*A and w = d2*B are written into
one [u|w] tile so ScalarE accumulates u^2+w^2 in a single Square pass per
tile; the final sqrt rescales by 0.25.

Strategy: pure batch data-parallelism (1024 elements/core), entity table
replicated in every core's HBM as one [200000, 1024] fp16 table whose rows
are [k=0 plane | k=1 plane].  Per core:
  1. sign-table precompute on device: s2, d2, a packed per relation as one
     fp16 [500, 1536] DRAM table ([s2|d2|a] rows).
  2. batch processed in 4 chunks of 2 tiles (2x128 elements).  All row
     gathers are [128,1]-offset indirect DMAs (the only offset layout the
     Q7 ucode supports - wider offset tensors silently gather consecutive
     rows): per chunk 2 head + 2 tail + 2 sign-table igathers, round-robin
     over the 4 SWDGE queues so 3+ queues drain concurrently (~360 GB/s
     aggregate vs ~150 GB/s for a single queue).  Gathers are issued two
     chunks ahead of compute.
  3. VectorE runs 9 fp16 tensor_tensor ops per chunk on [128,2,512]
     strided views (2 tiles fused per op to amortize the ~151-cycle DVE
     instruction setup).
  4. ScalarE: one Square+accumulate per tile over the packed [u|w] pair,
     final sqrt(0.25*(sum)).
Output [128, 8] f32 per core; host inverse-permutes to [8192].
"""

import sys

for _p in ("/opt/trn_rl_repo",):
    if _p not in sys.path:
        sys.path.insert(0, _p)

import numpy as np

import concourse.bass as bass
import concourse.bacc as bacc
import concourse.tile as tile
from concourse import mybir
from concourse.bass_utils import run_bass_kernel_spmd

NENTITY, NRELATION, EMB_DIM, K = 200000, 500, 512, 2
BATCH = 8192
NCORES = 8
B_LOC = BATCH // NCORES            # 1024 batch elements per core
NT = B_LOC // 128                  # 8 tiles of 128 per core
CH = 4                             # chunks per core
T_C = NT // CH                     # tiles per chunk (2)
CDT = mybir.dt.float16
NP_CDT = np.float16

F32 = mybir.dt.float32
F8 = mybir.dt.float8e4
I32 = mybir.dt.int32
AF = mybir.ActivationFunctionType
ALU = mybir.AluOpType

REL_P = 125
RROW = NRELATION // REL_P                    # 4 relation rows per partition
REL_FREE = NRELATION * EMB_DIM // REL_P      # 2048 (per plane)
SDA_W = 3 * EMB_DIM                          # 1536: [s2 | d2 | a]
LOOKAHEAD = 2                                # chunks of gather prefetch


def build_program():
    nc = bacc.Bacc("TRN2", target_bir_lowering=False, debug=False,
                   num_swdge_queues=4)

    ea = nc.declare_dram_parameter("ea", [NENTITY, 2 * EMB_DIM], CDT, isOutput=False)
    relx = nc.declare_dram_parameter("relx", [REL_P, REL_FREE], CDT, isOutput=False)
    rely = nc.declare_dram_parameter("rely", [REL_P, REL_FREE], CDT, isOutput=False)
    alphaf = nc.declare_dram_parameter("alphaf", [REL_P, REL_FREE], CDT, isOutput=False)
    htidx = nc.declare_dram_parameter("htidx", [128, 2 * NT], I32, isOutput=False)
    relidx = nc.declare_dram_parameter("relidx", [128, NT], I32, isOutput=False)
    out = nc.declare_dram_parameter("out", [128, NT], F32, isOutput=True)

    with tile.TileContext(nc) as tc:
        with (
            tc.tile_pool(name="dram", bufs=1, space="DRAM") as dramp,
            tc.tile_pool(name="idx", bufs=1) as idxp,
            tc.tile_pool(name="prep", bufs=1) as prep,
            tc.tile_pool(name="gat", bufs=4) as gat,
            tc.tile_pool(name="wrk", bufs=2) as wrk,
            tc.tile_pool(name="outp", bufs=1) as outp,
        ):
            # internal DRAM: per-relation [s2 | d2 | a] rows of 3*512 fp8
            # (sign values are exact in e4m3; halves write+gather bytes, the
            # cast back to fp16 happens inside the gather DMA)
            sda = dramp.tile([NRELATION, SDA_W], F8)

            # ---- index upload -------------------------------------------
            ht_t = idxp.tile([128, 2 * NT], I32)
            nc.sync.dma_start(out=ht_t[:], in_=htidx[:])
            rel_t = idxp.tile([128, NT], I32)
            nc.sync.dma_start(out=rel_t[:], in_=relidx[:])

            qn = [0]

            def igather(out_ap, in_ap, off_ap):
                inst = nc.gpsimd.indirect_dma_start(
                    out=out_ap, out_offset=None, in_=in_ap,
                    in_offset=bass.IndirectOffsetOnAxis(ap=off_ap, axis=0),
                )
                # first 12 calls avoid q3 (it carries the relation loads
                # and the sign-table write); the tail uses all 4 queues for
                # more concurrent drain
                q = qn[0] % 3 if qn[0] < 12 else qn[0] % 4
                qn[0] += 1
                if q:
                    inst.ins.queue = f"qPoolDynamic{q}"
                return inst

            hts = [None] * CH
            tts = [None] * CH
            gss = [None] * CH

            def issue_ht_gathers(c):
                gh = gat.tile([128, T_C, 2 * EMB_DIM], CDT, tag="gh")
                gt = gat.tile([128, T_C, 2 * EMB_DIM], CDT, tag="gt")
                for j in range(T_C):
                    t_idx = T_C * c + j
                    igather(gh[:, j, :], ea[:], ht_t[:, 2 * t_idx : 2 * t_idx + 1])
                    igather(gt[:, j, :], ea[:], ht_t[:, 2 * t_idx + 1 : 2 * t_idx + 2])
                hts[c] = gh
                tts[c] = gt

            def issue_sda_gathers(c):
                gs = gat.tile([128, T_C, SDA_W], CDT, tag="gs")
                for j in range(T_C):
                    t_idx = T_C * c + j
                    igather(gs[:, j, :], sda[:], rel_t[:, t_idx : t_idx + 1])
                gss[c] = gs

            # ---- sign-table precompute ----------------------------------
            rxsb = prep.tile([REL_P, REL_FREE], CDT)
            nc.scalar.dma_start(out=rxsb[:], in_=relx[:])
            rysb = prep.tile([REL_P, REL_FREE], CDT)
            i_ry = nc.gpsimd.dma_start(out=rysb[:], in_=rely[:])
            i_ry.ins.queue = "qPoolDynamic3"
            alsb = prep.tile([REL_P, REL_FREE], CDT)
            i_al = nc.gpsimd.dma_start(out=alsb[:], in_=alphaf[:])
            i_al.ins.queue = "qPoolDynamic3"
            sx = prep.tile([REL_P, REL_FREE], CDT)
            nc.scalar.activation(sx[:], rxsb[:], AF.Sign)
            sy = prep.tile([REL_P, REL_FREE], CDT)
            nc.scalar.activation(sy[:], rysb[:], AF.Sign)
            sda_sb = prep.tile([REL_P, RROW * SDA_W], CDT)
            sda_sbv = sda_sb[:].rearrange("p (r c d) -> p r c d", c=3, d=EMB_DIM)
            sx3 = sx[:].rearrange("p (r d) -> p r d", d=EMB_DIM)
            sy3 = sy[:].rearrange("p (r d) -> p r d", d=EMB_DIM)
            nc.vector.tensor_tensor(
                out=sda_sbv[:, :, 0, :], in0=sx3, in1=sy3, op=ALU.add
            )
            nc.vector.tensor_tensor(
                out=sda_sbv[:, :, 1, :], in0=sx3, in1=sy3, op=ALU.subtract
            )
            nc.scalar.activation(
                sda_sbv[:, :, 2, :],
                alsb[:].rearrange("p (r d) -> p r d", d=EMB_DIM),
                AF.Sign,
            )

            # preload the Sqrt LUT during the precompute window so the final
            # sqrt doesn't pay the ACT table swap on the critical tail
            sq_dummy = outp.tile([128, 1], F32)
            nc.vector.memset(sq_dummy[:], 1.0)
            nc.scalar.activation(sq_dummy[:], sq_dummy[:], AF.Sqrt)

            issue_ht_gathers(0)
            issue_ht_gathers(1)
            i_wr = nc.gpsimd.dma_start(
                out=sda[:].rearrange("(p r) w -> p (r w)", r=RROW), in_=sda_sb[:]
            )
            i_wr.ins.queue = "qPoolDynamic3"
            issue_sda_gathers(0)
            issue_sda_gathers(1)
            issue_ht_gathers(2)
            issue_sda_gathers(2)
            issue_ht_gathers(3)
            issue_sda_gathers(3)

            # ---- main loop ----------------------------------------------
            ssum = outp.tile([128, NT], F32)
            junk = outp.tile([128, 2 * EMB_DIM], CDT)

            def tt(tag, in0, in1, op, out_ap=None):
                if out_ap is None:
                    t = wrk.tile([128, T_C, EMB_DIM], CDT, tag=tag)
                    out_ap = t[:]
                nc.vector.tensor_tensor(out=out_ap, in0=in0, in1=in1, op=op)
                return out_ap

            for c in range(CH):
                gh3 = hts[c][:]
                gt3 = tts[c][:]
                gs3 = gss[c][:]
                h0 = gh3[:, :, 0:EMB_DIM]
                h1 = gh3[:, :, EMB_DIM : 2 * EMB_DIM]
                t0 = gt3[:, :, 0:EMB_DIM]
                t1v = gt3[:, :, EMB_DIM : 2 * EMB_DIM]
                s2v = gs3[:, :, 0:EMB_DIM]
                d2v = gs3[:, :, EMB_DIM : 2 * EMB_DIM]
                av = gs3[:, :, 2 * EMB_DIM : 3 * EMB_DIM]

                uw_t = wrk.tile([128, T_C, 2 * EMB_DIM], CDT, tag="uw")
                uw = uw_t[:]

                t1p = tt("t1p", av, t1v, ALU.mult)
                m1 = tt("m1", h0, t0, ALU.mult)
                m2 = tt("m2", h1, t1p, ALU.mult)
                A = tt("A", m1, m2, ALU.add)
                m4 = tt("m4", h1, t0, ALU.mult)
                m5 = tt("m5", h0, t1p, ALU.mult)
                B = tt("B", m4, m5, ALU.subtract)
                tt("u", s2v, A, ALU.mult, out_ap=uw[:, :, 0:EMB_DIM])
                tt("w", d2v, B, ALU.mult, out_ap=uw[:, :, EMB_DIM : 2 * EMB_DIM])

                for j in range(T_C):
                    t_idx = T_C * c + j
                    nc.scalar.activation(
                        junk[:], uw[:, j, :], AF.Square,
                        accum_out=ssum[:, t_idx : t_idx + 1],
                    )


            res = outp.tile([128, NT], F32)
            # score = sqrt(sum(term^2)) = sqrt(0.25 * sum((2*term)^2))
            nc.scalar.activation(res[:], ssum[:], AF.Sqrt, scale=0.25)
            nc.sync.dma_start(out=out[:], in_=res[:])

    nc.compile()
    return nc


_NC_CACHE = None


def _get_program():
    global _NC_CACHE
    if _NC_CACHE is None:
        _NC_CACHE = build_program()
    return _NC_CACHE


def make_in_maps(head_idx, relation_idx, tail_idx, entity_embedding,
                 relation_embedding, alpha_embedding):
    """Host-side sharding: slice batch 1024/core, replicate tables."""
    head_idx = np.asarray(head_idx).astype(np.int32)
    relation_idx = np.asarray(relation_idx).astype(np.int32)
    tail_idx = np.asarray(tail_idx).astype(np.int32)
    ent = np.asarray(entity_embedding)
    rel = np.asarray(relation_embedding)
    alp = np.asarray(alpha_embedding)

    # ea row r = [E[r,:,0,0] | E[r,:,0,1]]
    ea = np.ascontiguousarray(
        ent[:, :, 0, :].transpose(0, 2, 1).reshape(NENTITY, 2 * EMB_DIM)
    ).astype(NP_CDT)
    relx = np.ascontiguousarray(rel[:, :, 0]).astype(NP_CDT).reshape(REL_P, REL_FREE)
    rely = np.ascontiguousarray(rel[:, :, 1]).astype(NP_CDT).reshape(REL_P, REL_FREE)
    alphaf = alp.astype(NP_CDT).reshape(REL_P, REL_FREE)

    in_maps = []
    for c_core in range(NCORES):
        lo = c_core * B_LOC
        h = head_idx[lo : lo + B_LOC]
        tl = tail_idx[lo : lo + B_LOC]
        r = relation_idx[lo : lo + B_LOC]
        # htidx[p, 2t] = head of elem t*128+p ; [p, 2t+1] = tail
        htp = np.empty((128, 2 * NT), np.int32)
        for t in range(NT):
            htp[:, 2 * t] = h[128 * t : 128 * (t + 1)]
            htp[:, 2 * t + 1] = tl[128 * t : 128 * (t + 1)]
        rlp = np.empty((128, NT), np.int32)
        for t in range(NT):
            rlp[:, t] = r[128 * t : 128 * (t + 1)]
        in_maps.append(
            {
                "ea": ea,
                "relx": relx,
                "rely": rely,
                "alphaf": alphaf,
                "htidx": htp,
                "relidx": rlp,
            }
        )
    return in_maps


def unshard_out(results):
    """results: list of per-core dicts with 'out' [128, NT] f32."""
    full = np.empty(BATCH, np.float32)
    for c in range(NCORES):
        o = np.asarray(results[c]["out"])          # [128, NT], col = t
        # elem 128t + p  <-  o[p, t]
        full[c * B_LOC : (c + 1) * B_LOC] = o.T.ravel()
    return full


def kernel(head_idx, relation_idx, tail_idx, entity_embedding,
           relation_embedding, alpha_embedding):
    nc = _get_program()
    in_maps = make_in_maps(head_idx, relation_idx, tail_idx, entity_embedding,
                           relation_embedding, alpha_embedding)
    res = run_bass_kernel_spmd(nc, in_maps, list(range(NCORES)))
    return unshard_out(res.results)
